# revision 28
# baseline (speedup 1.0000x reference)
"""PointPillarScatter on 8 TRN2 cores via PE one-hot matmul (v3).

Scatter -> dense-matmul transform, [feat, col] orientation: host packs
pillars into 32-slot windows per 128-column tile; 4 tiles stack on the
128 SBUF partitions (tile j of a group at partitions [32j, 32j+32)).
Per group ONE one-hot P[slot, 512] = (colof'[slot] == iota512) is built
with a Vector/GpSimd tensor_scalar (colof' = tile-local column + 128*j,
so each tile owns a 128-col plane; empty slots hold -1), then ONE
matmul out[64 feats, 512 cols] = feats_g^T @ P_g covers 4 tiles
(fp16, 1 cyc/row, compact [128, 64] feats -- no block-diag inflation).

PSUM packing (hw rules found by probing: an accumulation group may not
span two column regions of a bank -- even chained start/stop faults at
exec -- but DISJOINT PARTITION ranges of one bank are independent
zero-regions): groups 2c, 2c+1 write partition halves [0:64) / [64:128)
of bank c%8.  One [128, 512] copy per chunk (Scalar/Vector) converts
fp32 PSUM -> fp16 stage.  Stages hold 2 chunks [128,1024] -> one
contiguous 256KB DMA each to a blocked HBM layout; host reorders and
upcasts to fp32.

fp16 end-to-end: the only error is fp16 rounding of the input features
(2^-11 rel); each value is moved exactly once (one-hot, no accums) and
fp32 PSUM -> fp16 stage -> host fp32 is exact for fp16 values.

Sharding: core k owns flat output columns [k*88000, (k+1)*88000) of the
5*140800 (cav, y, x) space; 688 tiles of 128 cols per core.
"""

import ml_dtypes
import numpy as np

import concourse.bass as bass
import concourse.tile as tile
from concourse import mybir
from concourse.bass_utils import run_bass_kernel_spmd

NUM_FEATURES = 64
MAX_CAV = 5
NX, NY = 704, 200
NUM_PIXELS = NY * NX            # 140800
TOTAL = MAX_CAV * NUM_PIXELS    # 704000
N_CORES = 8
CORE_COLS = TOTAL // N_CORES    # 88000 flat columns per core
TILE_COLS = 128
N_TILES = 688                   # 688*128 = 88064 >= 88000
SLOTS = 32                      # max pillars per tile (seed-0 max is 23)
TPG = 4                         # tiles per group (one matmul per group)
GROUPS = N_TILES // TPG         # 172
ROUNDS = (GROUPS + 7) // 8      # 22: 8 groups = 4 banks x 2 halves per round
FB = [0, 8, 60, 116, 172]       # feats const tile group boundaries (DMA split;
                                # small first tile so round 0 starts early)

FP16 = np.float16

_PROG = None


def _split_excess_waits(nc, max_waits=1):
    """Walrus enforces tight per-instruction sync-wait encoding limits. Spill
    surplus waits onto single-wait EventSemaphore nops inserted just before
    the offending instruction on the same engine queue (same semantics:
    engine blocks at the nop, then proceeds)."""
    for blk in nc.main_func.blocks:
        i = 0
        while i < len(blk.instructions):
            inst = blk.instructions[i]
            si = inst.sync_info
            if si is None or len(si.on_wait) <= max_waits:
                i += 1
                continue
            waits = list(si.on_wait)
            keep, spill = waits[-max_waits:], waits[:-max_waits]
            for w in spill:
                nop = mybir.InstEventSemaphore(
                    name=f"I-{nc.next_id()}", ins=[], outs=[]
                )
                nop.engine = inst.engine
                nop.sync_info = mybir.SyncInfo(on_wait=[w], on_update=[])
                nc.register_instruction(nop)
                blk.instructions.insert(i, nop)
                i += 1
            si.on_wait = keep
            inst.sync_info = si
            i += 1


def _build_prog():
    f32 = mybir.dt.float32
    fp16 = mybir.dt.float16
    nc = bass.Bass()
    feats = nc.dram_tensor("feats", [128, GROUPS * 64], fp16, kind="ExternalInput")
    colof = nc.dram_tensor("colof", [128, GROUPS], f32, kind="ExternalInput")
    # blocked output: row r*128 + ph*64 + f, col b*512 + jj*128 + x
    #   <-> tile (32r + 8b + 4ph + jj), column x, feature f
    out = nc.dram_tensor("out", [ROUNDS * 128, 2048], fp16, kind="ExternalOutput")

    with tile.TileContext(nc) as tc:
        with (
            tc.tile_pool(name="const", bufs=1) as constp,
            tc.tile_pool(name="pmat", bufs=8) as pmatp,
            tc.tile_pool(name="psum", bufs=2, space="PSUM") as psump,
            tc.tile_pool(name="stage", bufs=4) as stagep,
        ):
            iota_sb = constp.tile([128, 512], fp16)
            nc.gpsimd.iota(
                iota_sb[:], pattern=[[1, 512]], base=0,
                channel_multiplier=0, allow_small_or_imprecise_dtypes=True,
            )
            colof_sb = []
            feats_sb = []
            for q in range(len(FB) - 1):
                csb = constp.tile([128, FB[q + 1] - FB[q]], f32,
                                  name=f"colof{q}")
                nc.sync.dma_start(csb[:], colof[:, FB[q]:FB[q + 1]])
                colof_sb.append(csb)
                fsb = constp.tile([128, (FB[q + 1] - FB[q]) * 64], fp16,
                                  name=f"feats{q}")
                nc.sync.dma_start(
                    fsb[:], feats[:, FB[q] * 64:FB[q + 1] * 64]
                )
                feats_sb.append(fsb)

            def feats_slice(g):
                q = sum(1 for b in FB[1:] if b <= g)
                lo = g - FB[q]
                return feats_sb[q][:, lo * 64:(lo + 1) * 64]

            def colof_slice(g):
                q = sum(1 for b in FB[1:] if b <= g)
                lo = g - FB[q]
                return colof_sb[q][:, lo:lo + 1]

            # constant zero upper half of the final (2-bank) out row,
            # written during the head so it is off the drain tail
            zpad = constp.tile([128, 1024], fp16)
            nc.scalar.memzero(zpad[:])
            nc.sync.dma_start(
                out[(ROUNDS - 1) * 128:ROUNDS * 128, 1024:2048], zpad[:]
            )

            # PE pstate warmup: the PE ramps clock only under sustained
            # load; burn a few garbage matmuls while inputs stream in
            ps_w = psump.tile([128, 2048], f32, space="PSUM", name="ps")
            for w in range(8):
                nc.tensor.matmul(
                    out=ps_w[0:64, 256 * w:256 * (w + 1)],
                    lhsT=iota_sb[:, 0:64],
                    rhs=iota_sb[:, 0:256],
                    start=True,
                    stop=True,
                )

            for r in range(ROUNDS):
                st = stagep.tile([128, 2048], fp16)
                ps = psump.tile([128, 2048], f32, space="PSUM")
                nbank = min(4, (GROUPS - 8 * r + 1) // 2)   # last round: 2
                for b in range(nbank):
                    for ph in range(2):
                        g = 8 * r + 2 * b + ph
                        P = pmatp.tile([128, 512], fp16)
                        nc.vector.tensor_scalar(
                            P[:],
                            iota_sb[:],
                            colof_slice(g),
                            None,
                            mybir.AluOpType.is_equal,
                        )
                        nc.tensor.matmul(
                            out=ps[64 * ph:64 * (ph + 1),
                                   512 * b:512 * (b + 1)],
                            lhsT=feats_slice(g),
                            rhs=P[:],
                            start=True,
                            stop=True,
                        )
                w = 512 * nbank
                if r < ROUNDS - 1:
                    nc.scalar.copy(st[:, 0:w], ps[:, 0:w])
                    nc.sync.dma_start(out[r * 128:(r + 1) * 128, :], st[:])
                else:
                    # final round: bank-granular copy+DMA so the drain tail
                    # overlaps (each DMA flies while the next bank copies);
                    # the zero upper half was written during the head
                    orow = out[r * 128:(r + 1) * 128, :]
                    for b in range(nbank):
                        sl = slice(512 * b, 512 * (b + 1))
                        nc.scalar.copy(st[:, sl], ps[:, sl])
                        nc.sync.dma_start(orow[:, sl], st[:, sl])
    _split_excess_waits(nc)
    return nc


def _host_prep(voxel_coords, pillar_features):
    vc = voxel_coords.astype(np.int64)
    flat = vc[:, 0] * NUM_PIXELS + vc[:, 2] * NX + vc[:, 3]
    feats = np.ascontiguousarray(pillar_features, dtype=np.float32)
    core = flat // CORE_COLS
    rem = flat - core * CORE_COLS
    t = rem // TILE_COLS
    cof = rem - t * TILE_COLS
    jj = t % TPG
    grp = t // TPG
    # slot = rank of pillar within its (core, tile) group
    order = np.argsort(flat, kind="stable")
    gid_sorted = (core * N_TILES + t)[order]
    rank_sorted = np.arange(len(flat)) - np.searchsorted(
        gid_sorted, gid_sorted, side="left"
    )
    slot = np.empty(len(flat), np.int64)
    slot[order] = rank_sorted
    assert slot.max() < SLOTS, f"tile overflow: {slot.max() + 1} slots"
    row = SLOTS * jj + slot
    colp = cof + TILE_COLS * jj    # column within the group's 512-col plane

    in_maps = []
    for cidx in range(N_CORES):
        m = core == cidx
        fa = np.zeros((128, GROUPS, 64), FP16)
        ca = np.full((128, GROUPS), -1.0, np.float32)
        ca[row[m], grp[m]] = colp[m]
        fa[row[m], grp[m], :] = feats[m]
        in_maps.append({
            "feats": fa.reshape(128, GROUPS * 64),
            "colof": ca,
        })
    return in_maps


def _unshard(core_outs):
    full = np.empty((TOTAL, NUM_FEATURES), np.float32)
    for cidx, o in enumerate(core_outs):       # o: [22*128, 2048] fp16
        r = o.reshape(ROUNDS, 2, 64, 4, TPG, 128)   # [r, ph, f, b, jj, x]
        r = r.transpose(0, 3, 1, 4, 5, 2)           # [r, b, ph, jj, x, f]
        r = r.reshape(ROUNDS * 32 * 128, 64)
        full[cidx * CORE_COLS:(cidx + 1) * CORE_COLS] = r[:CORE_COLS]
    return np.ascontiguousarray(
        full.reshape(MAX_CAV, NUM_PIXELS, NUM_FEATURES)
        .transpose(0, 2, 1)
        .reshape(MAX_CAV, NUM_FEATURES, NY, NX)
    )


def kernel(voxel_coords, pillar_features):
    global _PROG
    if _PROG is None:
        _PROG = _build_prog()
    in_maps = _host_prep(voxel_coords, pillar_features)
    res = run_bass_kernel_spmd(_PROG, in_maps, list(range(N_CORES)))
    return _unshard([r["out"] for r in res.results])


# revision 29
# speedup vs baseline: 1.1071x; 1.1071x over previous
"""PointPillarScatter on 8 TRN2 cores via PE one-hot matmul (v3).

Scatter -> dense-matmul transform, [feat, col] orientation: host packs
pillars into 32-slot windows per 128-column tile; 4 tiles stack on the
128 SBUF partitions (tile j of a group at partitions [32j, 32j+32)).
Per group ONE one-hot P[slot, 512] = (colof'[slot] == iota512) is built
with a Vector/GpSimd tensor_scalar (colof' = tile-local column + 128*j,
so each tile owns a 128-col plane; empty slots hold -1), then ONE
matmul out[64 feats, 512 cols] = feats_g^T @ P_g covers 4 tiles
(fp16, 1 cyc/row, compact [128, 64] feats -- no block-diag inflation).

PSUM packing (hw rules found by probing: an accumulation group may not
span two column regions of a bank -- even chained start/stop faults at
exec -- but DISJOINT PARTITION ranges of one bank are independent
zero-regions): groups 2c, 2c+1 write partition halves [0:64) / [64:128)
of bank c%8.  One [128, 512] copy per chunk (Scalar/Vector) converts
fp32 PSUM -> fp16 stage.  Stages hold 2 chunks [128,1024] -> one
contiguous 256KB DMA each to a blocked HBM layout; host reorders and
upcasts to fp32.

fp16 end-to-end: the only error is fp16 rounding of the input features
(2^-11 rel); each value is moved exactly once (one-hot, no accums) and
fp32 PSUM -> fp16 stage -> host fp32 is exact for fp16 values.

Sharding: core k owns flat output columns [k*88000, (k+1)*88000) of the
5*140800 (cav, y, x) space; 688 tiles of 128 cols per core.
"""

import ml_dtypes
import numpy as np

import concourse.bass as bass
import concourse.tile as tile
from concourse import mybir
from concourse.bass_utils import run_bass_kernel_spmd

NUM_FEATURES = 64
MAX_CAV = 5
NX, NY = 704, 200
NUM_PIXELS = NY * NX            # 140800
TOTAL = MAX_CAV * NUM_PIXELS    # 704000
N_CORES = 8
CORE_COLS = TOTAL // N_CORES    # 88000 flat columns per core
TILE_COLS = 128
N_TILES = 688                   # 688*128 = 88064 >= 88000
SLOTS = 32                      # max pillars per tile (seed-0 max is 23)
TPG = 4                         # tiles per group (one matmul per group)
GROUPS = N_TILES // TPG         # 172
ROUNDS = (GROUPS + 7) // 8      # 22: 8 groups = 4 banks x 2 halves per round
FB = [0, 8, 60, 116, 172]       # feats const tile group boundaries (DMA split;
                                # small first tile so round 0 starts early)

FP16 = np.float16

_PROG = None


def _split_excess_waits(nc, max_waits=1):
    """Walrus enforces tight per-instruction sync-wait encoding limits. Spill
    surplus waits onto single-wait EventSemaphore nops inserted just before
    the offending instruction on the same engine queue (same semantics:
    engine blocks at the nop, then proceeds)."""
    for blk in nc.main_func.blocks:
        i = 0
        while i < len(blk.instructions):
            inst = blk.instructions[i]
            si = inst.sync_info
            if si is None or len(si.on_wait) <= max_waits:
                i += 1
                continue
            waits = list(si.on_wait)
            keep, spill = waits[-max_waits:], waits[:-max_waits]
            for w in spill:
                nop = mybir.InstEventSemaphore(
                    name=f"I-{nc.next_id()}", ins=[], outs=[]
                )
                nop.engine = inst.engine
                nop.sync_info = mybir.SyncInfo(on_wait=[w], on_update=[])
                nc.register_instruction(nop)
                blk.instructions.insert(i, nop)
                i += 1
            si.on_wait = keep
            inst.sync_info = si
            i += 1


def _build_prog():
    f32 = mybir.dt.float32
    fp16 = mybir.dt.float16
    nc = bass.Bass()
    feats = nc.dram_tensor("feats", [128, GROUPS * 64], fp16, kind="ExternalInput")
    colof = nc.dram_tensor("colof", [128, GROUPS], f32, kind="ExternalInput")
    # blocked output: row r*128 + ph*64 + f, col b*512 + jj*128 + x
    #   <-> tile (32r + 8b + 4ph + jj), column x, feature f
    out = nc.dram_tensor("out", [ROUNDS * 128, 2048], fp16, kind="ExternalOutput")

    with tile.TileContext(nc) as tc:
        with (
            tc.tile_pool(name="const", bufs=1) as constp,
            tc.tile_pool(name="pmat", bufs=8) as pmatp,
            tc.tile_pool(name="psum", bufs=2, space="PSUM") as psump,
            tc.tile_pool(name="stage", bufs=3) as stagep,
        ):
            iota_sb = constp.tile([128, 512], fp16)
            nc.gpsimd.iota(
                iota_sb[:], pattern=[[1, 512]], base=0,
                channel_multiplier=0, allow_small_or_imprecise_dtypes=True,
            )
            colof_sb = []
            feats_sb = []
            for q in range(len(FB) - 1):
                csb = constp.tile([128, FB[q + 1] - FB[q]], f32,
                                  name=f"colof{q}")
                nc.sync.dma_start(csb[:], colof[:, FB[q]:FB[q + 1]])
                colof_sb.append(csb)
                fsb = constp.tile([128, (FB[q + 1] - FB[q]) * 64], fp16,
                                  name=f"feats{q}")
                nc.sync.dma_start(
                    fsb[:], feats[:, FB[q] * 64:FB[q + 1] * 64]
                )
                feats_sb.append(fsb)

            def feats_slice(g):
                q = sum(1 for b in FB[1:] if b <= g)
                lo = g - FB[q]
                return feats_sb[q][:, lo * 64:(lo + 1) * 64]

            def colof_slice(g):
                q = sum(1 for b in FB[1:] if b <= g)
                lo = g - FB[q]
                return colof_sb[q][:, lo:lo + 1]

            # constant zero upper half of the final (2-bank) out row,
            # written during the head so it is off the drain tail
            zpad = constp.tile([128, 1024], fp16)
            nc.scalar.memzero(zpad[:])
            nc.sync.dma_start(
                out[(ROUNDS - 1) * 128:ROUNDS * 128, 1024:2048], zpad[:]
            )

            # PE pstate warmup: the PE ramps clock only under sustained
            # load; burn a few garbage matmuls while inputs stream in
            ps_w = psump.tile([128, 2048], f32, space="PSUM", name="ps")
            for w in range(4):
                nc.tensor.matmul(
                    out=ps_w[0:64, 512 * w:512 * (w + 1)],
                    lhsT=iota_sb[:, 0:64],
                    rhs=iota_sb[:],
                    start=True,
                    stop=True,
                )

            for r in range(ROUNDS):
                st = stagep.tile([128, 2048], fp16)
                ps = psump.tile([128, 2048], f32, space="PSUM")
                nbank = min(4, (GROUPS - 8 * r + 1) // 2)   # last round: 2
                for b in range(nbank):
                    for ph in range(2):
                        g = 8 * r + 2 * b + ph
                        P = pmatp.tile([128, 512], fp16)
                        nc.vector.tensor_scalar(
                            P[:],
                            iota_sb[:],
                            colof_slice(g),
                            None,
                            mybir.AluOpType.is_equal,
                        )
                        nc.tensor.matmul(
                            out=ps[64 * ph:64 * (ph + 1),
                                   512 * b:512 * (b + 1)],
                            lhsT=feats_slice(g),
                            rhs=P[:],
                            start=True,
                            stop=True,
                        )
                w = 512 * nbank
                if r < ROUNDS - 1:
                    nc.scalar.copy(st[:, 0:w], ps[:, 0:w])
                    nc.sync.dma_start(out[r * 128:(r + 1) * 128, :], st[:])
                else:
                    # final round: bank-granular copy+DMA so the drain tail
                    # overlaps (each DMA flies while the next bank copies);
                    # the zero upper half was written during the head
                    orow = out[r * 128:(r + 1) * 128, :]
                    for b in range(nbank):
                        sl = slice(512 * b, 512 * (b + 1))
                        nc.scalar.copy(st[:, sl], ps[:, sl])
                        nc.sync.dma_start(orow[:, sl], st[:, sl])
    _split_excess_waits(nc)
    return nc


def _host_prep(voxel_coords, pillar_features):
    vc = voxel_coords.astype(np.int64)
    flat = vc[:, 0] * NUM_PIXELS + vc[:, 2] * NX + vc[:, 3]
    feats = np.ascontiguousarray(pillar_features, dtype=np.float32)
    core = flat // CORE_COLS
    rem = flat - core * CORE_COLS
    t = rem // TILE_COLS
    cof = rem - t * TILE_COLS
    jj = t % TPG
    grp = t // TPG
    # slot = rank of pillar within its (core, tile) group
    order = np.argsort(flat, kind="stable")
    gid_sorted = (core * N_TILES + t)[order]
    rank_sorted = np.arange(len(flat)) - np.searchsorted(
        gid_sorted, gid_sorted, side="left"
    )
    slot = np.empty(len(flat), np.int64)
    slot[order] = rank_sorted
    assert slot.max() < SLOTS, f"tile overflow: {slot.max() + 1} slots"
    row = SLOTS * jj + slot
    colp = cof + TILE_COLS * jj    # column within the group's 512-col plane

    in_maps = []
    for cidx in range(N_CORES):
        m = core == cidx
        fa = np.zeros((128, GROUPS, 64), FP16)
        ca = np.full((128, GROUPS), -1.0, np.float32)
        ca[row[m], grp[m]] = colp[m]
        fa[row[m], grp[m], :] = feats[m]
        in_maps.append({
            "feats": fa.reshape(128, GROUPS * 64),
            "colof": ca,
        })
    return in_maps


def _unshard(core_outs):
    full = np.empty((TOTAL, NUM_FEATURES), np.float32)
    for cidx, o in enumerate(core_outs):       # o: [22*128, 2048] fp16
        r = o.reshape(ROUNDS, 2, 64, 4, TPG, 128)   # [r, ph, f, b, jj, x]
        r = r.transpose(0, 3, 1, 4, 5, 2)           # [r, b, ph, jj, x, f]
        r = r.reshape(ROUNDS * 32 * 128, 64)
        full[cidx * CORE_COLS:(cidx + 1) * CORE_COLS] = r[:CORE_COLS]
    return np.ascontiguousarray(
        full.reshape(MAX_CAV, NUM_PIXELS, NUM_FEATURES)
        .transpose(0, 2, 1)
        .reshape(MAX_CAV, NUM_FEATURES, NY, NX)
    )


def kernel(voxel_coords, pillar_features):
    global _PROG
    if _PROG is None:
        _PROG = _build_prog()
    in_maps = _host_prep(voxel_coords, pillar_features)
    res = run_bass_kernel_spmd(_PROG, in_maps, list(range(N_CORES)))
    return _unshard([r["out"] for r in res.results])


# revision 33
# speedup vs baseline: 1.1358x; 1.0259x over previous
"""PointPillarScatter on 8 TRN2 cores via PE one-hot matmul.

Scatter -> dense-matmul transform, [feat, col] orientation: host packs
pillars into 32-slot windows per 128-column tile; 4 tiles stack on the
128 SBUF partitions (tile j of a group at partitions [32j, 32j+32)).
Per group ONE one-hot P[slot, 512] = (colof'[slot] == iota512) is built
with a Vector tensor_scalar is_equal (colof' = tile-local column +
128*j so each tile owns a 128-col plane; empty slots hold -1; iota is
generated on-chip), then ONE matmul out[64 feats, 512 cols] =
feats_g^T @ P_g covers 4 tiles (fp16, 1 cyc/row, compact [128, 64]
feats -- no block-diag inflation).  The Vector engine's is_equal
stream (~267ns per 512-col group, DVE 2x 16-bit mode) is the
steady-state rate limiter; matmuls pipeline at the same cadence.

PSUM packing (hw rules found by probing: an accumulation group may not
span two column regions of a bank -- even chained start/stop faults at
exec -- but DISJOINT PARTITION ranges of one bank are independent
zero-regions): groups stack 2-per-bank on partition halves [0:64) /
[64:128), rounds of 4 banks fill a [128, 2048] psum tile (2 in
flight).  One Scalar copy [128, 2048] per round converts fp32 PSUM ->
fp16 stage; one contiguous 512KB DMA per round writes a blocked HBM
layout; host reorders and upcasts to fp32.  GpSimd is avoided for
compute: its TensorScalar ucode runs ~8us/op and TensorTensor is not
in the Pool ISA; it only generates the iota constant.

Head/tail: inputs are DMAd in interleaved chunks (small first chunk so
round 0 starts early), 4 garbage matmuls warm the PE clock ramp, the
final round's constant zero half is written during the head and its
copies/DMAs are bank-granular to shorten the drain.  ~15us of the
remaining runtime is fixed preamble/epilogue (engine table loads and a
per-semaphore NRT teardown chain appended by the NEFF finalizer).

fp16 end-to-end: the only error is fp16 rounding of the input features
(2^-11 rel); each value is moved exactly once (one-hot, no accums) and
fp32 PSUM -> fp16 stage -> host fp32 is exact for fp16 values.

Sharding: core k owns flat output columns [k*88000, (k+1)*88000) of the
5*140800 (cav, y, x) space; 688 tiles of 128 cols per core.
"""

import numpy as np

import concourse.bass as bass
import concourse.tile as tile
from concourse import mybir
from concourse.bass_utils import run_bass_kernel_spmd

NUM_FEATURES = 64
MAX_CAV = 5
NX, NY = 704, 200
NUM_PIXELS = NY * NX            # 140800
TOTAL = MAX_CAV * NUM_PIXELS    # 704000
N_CORES = 8
CORE_COLS = TOTAL // N_CORES    # 88000 flat columns per core
TILE_COLS = 128
N_TILES = 688                   # 688*128 = 88064 >= 88000
SLOTS = 32                      # max pillars per tile (seed-0 max is 23)
TPG = 4                         # tiles per group (one matmul per group)
GROUPS = N_TILES // TPG         # 172
ROUNDS = (GROUPS + 7) // 8      # 22: 8 groups = 4 banks x 2 halves per round
FB = [0, 8, 60, 116, 172]       # feats const tile group boundaries (DMA split;
                                # small first tile so round 0 starts early)

FP16 = np.float16

_PROG = None


def _split_excess_waits(nc, max_waits=1):
    """Walrus enforces tight per-instruction sync-wait encoding limits. Spill
    surplus waits onto single-wait EventSemaphore nops inserted just before
    the offending instruction on the same engine queue (same semantics:
    engine blocks at the nop, then proceeds)."""
    for blk in nc.main_func.blocks:
        i = 0
        while i < len(blk.instructions):
            inst = blk.instructions[i]
            si = inst.sync_info
            if si is None or len(si.on_wait) <= max_waits:
                i += 1
                continue
            waits = list(si.on_wait)
            keep, spill = waits[-max_waits:], waits[:-max_waits]
            for w in spill:
                nop = mybir.InstEventSemaphore(
                    name=f"I-{nc.next_id()}", ins=[], outs=[]
                )
                nop.engine = inst.engine
                nop.sync_info = mybir.SyncInfo(on_wait=[w], on_update=[])
                nc.register_instruction(nop)
                blk.instructions.insert(i, nop)
                i += 1
            si.on_wait = keep
            inst.sync_info = si
            i += 1


def _strip_end_block(nc):
    """Drop the second quiesce set (drain+barrier per engine), the gpsimd
    semaphore range-clear, and the SP terminal-value waits from the teardown
    block: NRT re-establishes semaphore state per execution, and the first
    drain+barrier set already quiesces every engine."""
    blk = nc.main_func.blocks[-1]
    insts = blk.instructions
    # find the Pool ISA RANGE_CLEAR; keep everything up to and including the
    # first barrier set that precedes it, drop the clear and the second set
    cut = None
    for idx, inst in enumerate(insts):
        if type(inst).__name__ == "InstISA":
            cut = idx
            break
    if cut is not None:
        del insts[cut:]


def _build_prog():
    f32 = mybir.dt.float32
    fp16 = mybir.dt.float16
    nc = bass.Bass()
    feats = nc.dram_tensor("feats", [128, GROUPS * 64], fp16, kind="ExternalInput")
    colof = nc.dram_tensor("colof", [128, GROUPS], f32, kind="ExternalInput")
    # blocked output: row r*128 + ph*64 + f, col b*512 + jj*128 + x
    #   <-> tile (32r + 8b + 4ph + jj), column x, feature f
    out = nc.dram_tensor("out", [ROUNDS * 128, 2048], fp16, kind="ExternalOutput")

    with tile.TileContext(nc) as tc:
        with (
            tc.tile_pool(name="const", bufs=1) as constp,
            tc.tile_pool(name="pmat", bufs=8) as pmatp,
            tc.tile_pool(name="psum", bufs=2, space="PSUM") as psump,
            tc.tile_pool(name="stage", bufs=3) as stagep,
        ):
            iota_sb = constp.tile([128, 512], fp16)
            nc.gpsimd.iota(
                iota_sb[:], pattern=[[1, 512]], base=0,
                channel_multiplier=0, allow_small_or_imprecise_dtypes=True,
            )
            colof_sb = []
            feats_sb = []
            for q in range(len(FB) - 1):
                csb = constp.tile([128, FB[q + 1] - FB[q]], f32,
                                  name=f"colof{q}")
                nc.sync.dma_start(csb[:], colof[:, FB[q]:FB[q + 1]])
                colof_sb.append(csb)
                fsb = constp.tile([128, (FB[q + 1] - FB[q]) * 64], fp16,
                                  name=f"feats{q}")
                nc.sync.dma_start(
                    fsb[:], feats[:, FB[q] * 64:FB[q + 1] * 64]
                )
                feats_sb.append(fsb)

            def feats_slice(g):
                q = sum(1 for b in FB[1:] if b <= g)
                lo = g - FB[q]
                return feats_sb[q][:, lo * 64:(lo + 1) * 64]

            def colof_slice(g):
                q = sum(1 for b in FB[1:] if b <= g)
                lo = g - FB[q]
                return colof_sb[q][:, lo:lo + 1]

            # constant zero upper half of the final (2-bank) out row,
            # written during the head so it is off the drain tail
            zpad = constp.tile([128, 1024], fp16)
            nc.scalar.memzero(zpad[:])
            nc.sync.dma_start(
                out[(ROUNDS - 1) * 128:ROUNDS * 128, 1024:2048], zpad[:]
            )

            # PE pstate warmup: the PE ramps clock only under sustained
            # load; burn a few garbage matmuls while inputs stream in
            ps_w = psump.tile([128, 2048], f32, space="PSUM", name="ps")
            for w in range(4):
                nc.tensor.matmul(
                    out=ps_w[0:64, 512 * w:512 * (w + 1)],
                    lhsT=iota_sb[:, 0:64],
                    rhs=iota_sb[:],
                    start=True,
                    stop=True,
                )

            for r in range(ROUNDS):
                st = stagep.tile([128, 2048], fp16)
                ps = psump.tile([128, 2048], f32, space="PSUM")
                nbank = min(4, (GROUPS - 8 * r + 1) // 2)   # last round: 2
                for b in range(nbank):
                    for ph in range(2):
                        g = 8 * r + 2 * b + ph
                        P = pmatp.tile([128, 512], fp16)
                        nc.vector.tensor_scalar(
                            P[:],
                            iota_sb[:],
                            colof_slice(g),
                            None,
                            mybir.AluOpType.is_equal,
                        )
                        nc.tensor.matmul(
                            out=ps[64 * ph:64 * (ph + 1),
                                   512 * b:512 * (b + 1)],
                            lhsT=feats_slice(g),
                            rhs=P[:],
                            start=True,
                            stop=True,
                        )
                w = 512 * nbank
                if r < ROUNDS - 1:
                    nc.scalar.copy(st[:, 0:w], ps[:, 0:w])
                    nc.sync.dma_start(out[r * 128:(r + 1) * 128, :], st[:])
                else:
                    # final round: bank-granular copy+DMA so the drain tail
                    # overlaps (each DMA flies while the next bank copies);
                    # the zero upper half was written during the head
                    orow = out[r * 128:(r + 1) * 128, :]
                    for b in range(nbank):
                        sl = slice(512 * b, 512 * (b + 1))
                        nc.scalar.copy(st[:, sl], ps[:, sl])
                        nc.sync.dma_start(orow[:, sl], st[:, sl])
    _split_excess_waits(nc)
    _strip_end_block(nc)
    return nc


def _host_prep(voxel_coords, pillar_features):
    vc = voxel_coords.astype(np.int64)
    flat = vc[:, 0] * NUM_PIXELS + vc[:, 2] * NX + vc[:, 3]
    feats = np.ascontiguousarray(pillar_features, dtype=np.float32)
    core = flat // CORE_COLS
    rem = flat - core * CORE_COLS
    t = rem // TILE_COLS
    cof = rem - t * TILE_COLS
    jj = t % TPG
    grp = t // TPG
    # slot = rank of pillar within its (core, tile) group
    order = np.argsort(flat, kind="stable")
    gid_sorted = (core * N_TILES + t)[order]
    rank_sorted = np.arange(len(flat)) - np.searchsorted(
        gid_sorted, gid_sorted, side="left"
    )
    slot = np.empty(len(flat), np.int64)
    slot[order] = rank_sorted
    assert slot.max() < SLOTS, f"tile overflow: {slot.max() + 1} slots"
    row = SLOTS * jj + slot
    colp = cof + TILE_COLS * jj    # column within the group's 512-col plane

    in_maps = []
    for cidx in range(N_CORES):
        m = core == cidx
        fa = np.zeros((128, GROUPS, 64), FP16)
        ca = np.full((128, GROUPS), -1.0, np.float32)
        ca[row[m], grp[m]] = colp[m]
        fa[row[m], grp[m], :] = feats[m]
        in_maps.append({
            "feats": fa.reshape(128, GROUPS * 64),
            "colof": ca,
        })
    return in_maps


def _unshard(core_outs):
    full = np.empty((TOTAL, NUM_FEATURES), np.float32)
    for cidx, o in enumerate(core_outs):       # o: [22*128, 2048] fp16
        r = o.reshape(ROUNDS, 2, 64, 4, TPG, 128)   # [r, ph, f, b, jj, x]
        r = r.transpose(0, 3, 1, 4, 5, 2)           # [r, b, ph, jj, x, f]
        r = r.reshape(ROUNDS * 32 * 128, 64)
        full[cidx * CORE_COLS:(cidx + 1) * CORE_COLS] = r[:CORE_COLS]
    return np.ascontiguousarray(
        full.reshape(MAX_CAV, NUM_PIXELS, NUM_FEATURES)
        .transpose(0, 2, 1)
        .reshape(MAX_CAV, NUM_FEATURES, NY, NX)
    )


def kernel(voxel_coords, pillar_features):
    global _PROG
    if _PROG is None:
        _PROG = _build_prog()
    in_maps = _host_prep(voxel_coords, pillar_features)
    res = run_bass_kernel_spmd(_PROG, in_maps, list(range(N_CORES)))
    return _unshard([r["out"] for r in res.results])
